# revision 16
# baseline (speedup 1.0000x reference)
"""Trainium2 Bass kernel for DigitConvolutionalModel (conv3x3 -> FC512 -> FC10).

Math: the 3x3 valid conv is linear, so  y_flat = x @ C  with C [784, 676]
holding conv_w values in a banded structure.  Then
    logits = relu(x @ (C @ W1) + b1) @ W2 + b2
The fold W1_eff = C @ W1 is computed on device (banded matmul over only
the nonzero blocks), then the big [2048, 784] @ [784, 512] matmul per
core, relu, and the [*, 512] @ [512, 10] head.  Data-parallel across 8
cores on the batch dim.

Host-side work is layout/dtype only: shard + transpose x, cast to bf16,
arrange C^T / W1 / b1 / W2 / b2 into SBUF-friendly layouts.
"""

import numpy as np
import ml_dtypes

B = 16384
IMG = 28
K = 3
OUT = IMG - K + 1  # 26
M26 = OUT * OUT  # 676
Q = IMG * IMG  # 784
HID = 512
NCLS = 10

NCORES = 8
BL = B // NCORES  # 2048 rows per core
QT = 112  # q-tile height (partition dim), 7 tiles
NQT = Q // QT  # 7
SB = 512  # batch superblock (matmul N)
NSB = BL // SB  # 4
NHT = HID // 128  # 4
NMC = (M26 + 127) // 128  # 6 m-chunks
NWARM = 8  # dummy matmuls to warm the PE/HAM during the DMA prologue

TRACE = False  # set by test harness to capture an NTFF profile
_CACHE = {}

_BF16 = ml_dtypes.bfloat16


def _band_blocks():
    """Static nonzero block pattern of C^T [676, 784] against (mc, qt) tiling.

    Returns per q-tile the list of m-chunk indices whose [128, QT] block of
    C^T contains structural nonzeros.
    """
    Cs = np.zeros((Q, M26), dtype=bool)
    ii, jj = np.meshgrid(np.arange(OUT), np.arange(OUT), indexing="ij")
    m = (OUT * ii + jj).ravel()
    for di in range(K):
        for dj in range(K):
            q = ((ii + di) * IMG + (jj + dj)).ravel()
            Cs[q, m] = True
    CT = Cs.T  # [676, 784]
    blocks = []
    for t in range(NQT):
        mcs = []
        for mc in range(NMC):
            rows = min(128, M26 - 128 * mc)
            if CT[128 * mc : 128 * mc + rows, QT * t : QT * (t + 1)].any():
                mcs.append(mc)
        blocks.append(mcs)
    return blocks


_BLOCKS = _band_blocks()
# flat list of (t, mc) pairs; the packed cmat input carries one [128, QT]
# block per pair, in this order
_PAIRS = [(t, mc) for t in range(NQT) for mc in _BLOCKS[t]]
NP_ = len(_PAIRS)


def _build():
    import concourse.bacc as bacc
    import concourse.mybir as mybir
    import concourse.tile as tile

    f32 = mybir.dt.float32
    bf16 = mybir.dt.bfloat16
    AF = mybir.ActivationFunctionType

    nc = bacc.Bacc("TRN2", target_bir_lowering=False, debug=False)

    # packed weights: one wide row per partition so every DMA moves
    # multi-KB contiguous chunks
    xt_d = nc.dram_tensor("xt", [Q, BL], bf16, kind="ExternalInput")
    cm_d = nc.dram_tensor("cmbl", [128, NP_ * QT], bf16, kind="ExternalInput")
    w1_d = nc.dram_tensor("w1p", [128, NMC * HID], bf16, kind="ExternalInput")
    b1_d = nc.dram_tensor("b1l", [128, NHT], f32, kind="ExternalInput")
    w2_d = nc.dram_tensor("w2l", [128, NHT * NCLS], bf16, kind="ExternalInput")
    b2_d = nc.dram_tensor("b2l", [NCLS, 1], f32, kind="ExternalInput")
    out_d = nc.dram_tensor("out", [NCLS, BL], f32, kind="ExternalOutput")

    with tile.TileContext(nc) as tc:
        with (
            tc.tile_pool(name="weights", bufs=1) as wp,
            tc.tile_pool(name="xin", bufs=1) as xp,
            tc.tile_pool(name="hid", bufs=2) as hp,
            tc.tile_pool(name="lgts", bufs=2) as lp,
            tc.tile_pool(name="psF", bufs=2, space="PSUM") as psF,
            tc.tile_pool(name="ps1", bufs=1, space="PSUM") as ps1p,
            tc.tile_pool(name="ps2", bufs=2, space="PSUM") as ps2p,
        ):
            # ---- PE warmup: dependency-light matmuls on scratch data ----
            # They issue as soon as the tiny memset lands, keeping the PE
            # busy through the weight-DMA prologue so HAM reaches K=8/8
            # before real work starts.  Results are never read.
            scratch = wp.tile([128, HID], bf16, tag="scratch")
            nc.gpsimd.memset(scratch[:], 0.0)
            warm = psF.tile([128, HID], f32, tag="ps")
            for i in range(NWARM):
                nc.tensor.matmul(
                    warm[:],
                    lhsT=scratch[:, :128],
                    rhs=scratch[:],
                    start=True,
                    stop=True,
                )

            # ---- sync-queue DMAs, in need order: fold weights, then x
            # (one big transfer per q-tile), then the late-needed weights.
            # Few, large transfers: each DMA trigger occupies the issuing
            # queue ~650ns, so trigger count is itself a bottleneck.
            cmb = wp.tile([128, NP_ * QT], bf16, tag="cmb")
            nc.sync.dma_start(out=cmb[:], in_=cm_d[:, :])
            w1p = wp.tile([128, NMC * HID], bf16, tag="w1p")
            nc.sync.dma_start(out=w1p[:], in_=w1_d[:, :])
            xts = []
            for t in range(NQT):
                xt = xp.tile([QT, BL], bf16, tag=f"x_{t}", name=f"x_{t}")
                nc.sync.dma_start(out=xt[:], in_=xt_d[QT * t : QT * (t + 1), :])
                xts.append(xt)
            b1 = wp.tile([128, NHT], f32, tag="b1")
            nc.scalar.dma_start(out=b1[:], in_=b1_d[:, :])
            w2 = wp.tile([128, NHT * NCLS], bf16, tag="w2")
            nc.scalar.dma_start(out=w2[:], in_=w2_d[:, :])
            b2 = wp.tile([NCLS, 1], f32, tag="b2")
            nc.scalar.dma_start(out=b2[:], in_=b2_d[:, :])

            # ---- fold: W1_eff[q, h] = sum_m C^T[m, q] * W1[m, h] ----
            pair_idx = {pair: i for i, pair in enumerate(_PAIRS)}
            w1eff = []
            for t in range(NQT):
                ps = psF.tile([QT, HID], f32, tag="ps", name=f"foldps_{t}")
                mcs = _BLOCKS[t]
                for j, mc in enumerate(mcs):
                    rows = min(128, M26 - 128 * mc)
                    p = pair_idx[(t, mc)]
                    nc.tensor.matmul(
                        ps[:],
                        lhsT=cmb[:rows, QT * p : QT * (p + 1)],
                        rhs=w1p[:rows, HID * mc : HID * (mc + 1)],
                        start=(j == 0),
                        stop=(j == len(mcs) - 1),
                    )
                we = wp.tile([QT, HID], bf16, tag=f"we{t}", name=f"we{t}")
                nc.vector.tensor_copy(we[:], ps[:])
                w1eff.append(we)

            # ---- main loop over batch superblocks ----
            for s in range(NSB):
                ps1s = [
                    ps1p.tile([128, SB], f32, tag=f"ps1_{ht}", name=f"ps1_{ht}")
                    for ht in range(NHT)
                ]
                hs = []
                if s == 0:
                    # t-outer: each x tile feeds 4 matmuls as soon as its DMA
                    # lands (covers the input ramp)
                    for t in range(NQT):
                        for ht in range(NHT):
                            nc.tensor.matmul(
                                ps1s[ht][:],
                                lhsT=w1eff[t][:, 128 * ht : 128 * (ht + 1)],
                                rhs=xts[t][:, SB * s : SB * (s + 1)],
                                start=(t == 0),
                                stop=(t == NQT - 1),
                            )
                    for ht in range(NHT):
                        h = hp.tile([128, SB], bf16, tag=f"h{ht}", name=f"h{ht}")
                        nc.scalar.activation(
                            h[:],
                            ps1s[ht][:],
                            AF.Relu,
                            bias=b1[:, ht : ht + 1],
                            scale=1.0,
                        )
                        hs.append(h)
                else:
                    # ht-outer: each relu fires right after its group, so the
                    # next superblock's matmuls never wait on a burst of ACTs
                    for ht in range(NHT):
                        for t in range(NQT):
                            nc.tensor.matmul(
                                ps1s[ht][:],
                                lhsT=w1eff[t][:, 128 * ht : 128 * (ht + 1)],
                                rhs=xts[t][:, SB * s : SB * (s + 1)],
                                start=(t == 0),
                                stop=(t == NQT - 1),
                            )
                        h = hp.tile([128, SB], bf16, tag=f"h{ht}", name=f"h{ht}")
                        nc.scalar.activation(
                            h[:],
                            ps1s[ht][:],
                            AF.Relu,
                            bias=b1[:, ht : ht + 1],
                            scale=1.0,
                        )
                        hs.append(h)
                ps2 = ps2p.tile([NCLS, SB], f32)
                for ht in range(NHT):
                    nc.tensor.matmul(
                        ps2[:],
                        lhsT=w2[:, NCLS * ht : NCLS * (ht + 1)],
                        rhs=hs[ht][:],
                        start=(ht == 0),
                        stop=(ht == NHT - 1),
                    )
                lg = lp.tile([NCLS, SB], f32, tag="lg")
                nc.vector.tensor_scalar(
                    lg[:], ps2[:], b2[:, 0:1], None, mybir.AluOpType.add
                )
                nc.sync.dma_start(out=out_d[:, SB * s : SB * (s + 1)], in_=lg[:])

    nc.compile()
    return nc


def _get_nc():
    if "nc" not in _CACHE:
        _CACHE["nc"] = _build()
    return _CACHE["nc"]


def kernel(x, conv_w, W1, b1, W2, b2):
    from concourse.bass_utils import run_bass_kernel_spmd

    nc = _get_nc()

    # C [784, 676]: y_flat = x @ C  (banded placement of conv_w values)
    C = np.zeros((Q, M26), dtype=np.float32)
    ii, jj = np.meshgrid(np.arange(OUT), np.arange(OUT), indexing="ij")
    m = (OUT * ii + jj).ravel()
    cw = np.asarray(conv_w, dtype=np.float32)
    for di in range(K):
        for dj in range(K):
            q = ((ii + di) * IMG + (jj + dj)).ravel()
            C[q, m] = cw[di, dj]
    CT = C.T  # [676, 784]
    # packed banded blocks: cmbl[p, 112*i : 112*(i+1)] = block i, row p
    cmbl = np.zeros((128, NP_ * QT), dtype=np.float32)
    for p, (t, mc) in enumerate(_PAIRS):
        rows = min(128, M26 - 128 * mc)
        cmbl[:rows, QT * p : QT * (p + 1)] = CT[
            128 * mc : 128 * mc + rows, QT * t : QT * (t + 1)
        ]
    cmbl = cmbl.astype(_BF16)

    # packed W1: w1p[p, 512*mc : 512*(mc+1)] = W1[128*mc + p, :]
    w1f = np.asarray(W1, np.float32)
    w1p = np.zeros((128, NMC * HID), dtype=np.float32)
    for mc in range(NMC):
        rows = min(128, M26 - 128 * mc)
        w1p[:rows, HID * mc : HID * (mc + 1)] = w1f[128 * mc : 128 * mc + rows, :]
    w1p = w1p.astype(_BF16)

    b1l = np.ascontiguousarray(
        np.asarray(b1, np.float32).reshape(NHT, 128).T
    )  # [128, 4]
    w2l = np.ascontiguousarray(
        np.asarray(W2, np.float32)
        .reshape(NHT, 128, NCLS)
        .transpose(1, 0, 2)
        .reshape(128, NHT * NCLS)
    ).astype(_BF16)
    b2l = np.asarray(b2, np.float32).reshape(NCLS, 1)

    xf = np.asarray(x, np.float32)
    in_maps = []
    for c in range(NCORES):
        xt = np.ascontiguousarray(xf[c * BL : (c + 1) * BL].T).astype(_BF16)
        in_maps.append(
            {
                "xt": xt,
                "cmbl": cmbl,
                "w1p": w1p,
                "b1l": b1l,
                "w2l": w2l,
                "b2l": b2l,
            }
        )

    kwargs = {}
    if TRACE:
        import profhook  # noqa: F401  (installs the NTFF hook shim)
        import tempfile

        kwargs = {"trace": True, "tmpdir": tempfile.mkdtemp(prefix="ntff_")}
    res = run_bass_kernel_spmd(nc, in_maps, core_ids=list(range(NCORES)), **kwargs)
    if TRACE:
        _CACHE["last_results"] = res

    out = np.concatenate(
        [np.ascontiguousarray(res.results[c]["out"].T) for c in range(NCORES)], axis=0
    ).astype(np.float32)
    return out


# revision 19
# speedup vs baseline: 1.0313x; 1.0313x over previous
"""Trainium2 Bass kernel for DigitConvolutionalModel (conv3x3 -> FC512 -> FC10).

Math: the 3x3 valid conv is linear, so  y_flat = x @ C  with C [784, 676]
holding conv_w values in a banded structure.  Then
    logits = relu(x @ (C @ W1) + b1) @ W2 + b2
The fold W1_eff = C @ W1 is computed on device (banded matmul over only
the nonzero blocks), then the big [2048, 784] @ [784, 512] matmul per
core, relu, and the [*, 512] @ [512, 10] head.  Data-parallel across 8
cores on the batch dim.

Host-side work is layout/dtype only: shard + transpose x, cast to bf16,
arrange C^T / W1 / b1 / W2 / b2 into SBUF-friendly layouts.
"""

import numpy as np
import ml_dtypes

B = 16384
IMG = 28
K = 3
OUT = IMG - K + 1  # 26
M26 = OUT * OUT  # 676
Q = IMG * IMG  # 784
HID = 512
NCLS = 10

NCORES = 8
BL = B // NCORES  # 2048 rows per core
QT = 112  # q-tile height (partition dim), 7 tiles
NQT = Q // QT  # 7
SB = 512  # batch superblock (matmul N)
NSB = BL // SB  # 4
NHT = HID // 128  # 4
NMC = (M26 + 127) // 128  # 6 m-chunks
NWARM = 8  # dummy matmuls to warm the PE/HAM during the DMA prologue

TRACE = False  # set by test harness to capture an NTFF profile
_CACHE = {}

_BF16 = ml_dtypes.bfloat16


def _band_blocks():
    """Static nonzero block pattern of C^T [676, 784] against (mc, qt) tiling.

    Returns per q-tile the list of m-chunk indices whose [128, QT] block of
    C^T contains structural nonzeros.
    """
    Cs = np.zeros((Q, M26), dtype=bool)
    ii, jj = np.meshgrid(np.arange(OUT), np.arange(OUT), indexing="ij")
    m = (OUT * ii + jj).ravel()
    for di in range(K):
        for dj in range(K):
            q = ((ii + di) * IMG + (jj + dj)).ravel()
            Cs[q, m] = True
    CT = Cs.T  # [676, 784]
    blocks = []
    for t in range(NQT):
        mcs = []
        for mc in range(NMC):
            rows = min(128, M26 - 128 * mc)
            if CT[128 * mc : 128 * mc + rows, QT * t : QT * (t + 1)].any():
                mcs.append(mc)
        blocks.append(mcs)
    return blocks


_BLOCKS = _band_blocks()
# flat list of (t, mc) pairs; the packed cmat input carries one [128, QT]
# block per pair, in this order
_PAIRS = [(t, mc) for t in range(NQT) for mc in _BLOCKS[t]]
NP_ = len(_PAIRS)


def _build():
    import concourse.bacc as bacc
    import concourse.mybir as mybir
    import concourse.tile as tile

    f32 = mybir.dt.float32
    bf16 = mybir.dt.bfloat16
    AF = mybir.ActivationFunctionType

    nc = bacc.Bacc("TRN2", target_bir_lowering=False, debug=False)

    # packed weights: one wide row per partition so every DMA moves
    # multi-KB contiguous chunks
    xt_d = nc.dram_tensor("xt", [Q, BL], bf16, kind="ExternalInput")
    cm_d = nc.dram_tensor("cmbl", [128, NP_ * QT], bf16, kind="ExternalInput")
    w1_d = nc.dram_tensor("w1p", [128, NMC * HID], bf16, kind="ExternalInput")
    b1_d = nc.dram_tensor("b1l", [128, NHT], f32, kind="ExternalInput")
    w2_d = nc.dram_tensor("w2l", [128, NHT * NCLS], bf16, kind="ExternalInput")
    b2_d = nc.dram_tensor("b2l", [NCLS, 1], f32, kind="ExternalInput")
    out_d = nc.dram_tensor("out", [NCLS, BL], f32, kind="ExternalOutput")

    with tile.TileContext(nc) as tc:
        with (
            tc.tile_pool(name="weights", bufs=1) as wp,
            tc.tile_pool(name="xin", bufs=1) as xp,
            tc.tile_pool(name="hid", bufs=2) as hp,
            tc.tile_pool(name="lgts", bufs=2) as lp,
            tc.tile_pool(name="psF", bufs=2, space="PSUM") as psF,
            tc.tile_pool(name="ps1", bufs=1, space="PSUM") as ps1p,
            tc.tile_pool(name="ps2", bufs=2, space="PSUM") as ps2p,
        ):
            # ---- PE warmup: dependency-light matmuls on scratch data ----
            # They issue as soon as the tiny memset lands, keeping the PE
            # busy through the weight-DMA prologue so HAM reaches K=8/8
            # before real work starts.  Results are never read.
            scratch = wp.tile([128, HID], bf16, tag="scratch")
            nc.gpsimd.memset(scratch[:], 0.0)
            warm = psF.tile([128, HID], f32, tag="ps")
            for i in range(NWARM):
                nc.tensor.matmul(
                    warm[:],
                    lhsT=scratch[:, :128],
                    rhs=scratch[:],
                    start=True,
                    stop=True,
                )

            # ---- sync-queue DMAs, in need order: fold weights, then x
            # (one big transfer per q-tile), then the late-needed weights.
            # Few, large transfers: each DMA trigger occupies the issuing
            # queue ~650ns, so trigger count is itself a bottleneck.
            cmb = wp.tile([128, NP_ * QT], bf16, tag="cmb")
            nc.sync.dma_start(out=cmb[:], in_=cm_d[:, :])
            w1p = wp.tile([128, NMC * HID], bf16, tag="w1p")
            nc.sync.dma_start(out=w1p[:], in_=w1_d[:, :])
            # s0 slices first (small, land before the fold finishes), then
            # the bulk for s1..s3
            xs0, xr = [], []
            for t in range(NQT):
                x0 = xp.tile([QT, SB], bf16, tag=f"xa_{t}", name=f"xa_{t}")
                nc.sync.dma_start(out=x0[:], in_=xt_d[QT * t : QT * (t + 1), 0:SB])
                xs0.append(x0)
            for t in range(NQT):
                xb = xp.tile([QT, BL - SB], bf16, tag=f"xb_{t}", name=f"xb_{t}")
                nc.sync.dma_start(out=xb[:], in_=xt_d[QT * t : QT * (t + 1), SB:BL])
                xr.append(xb)

            def xslice(s, t):
                if s == 0:
                    return xs0[t][:]
                return xr[t][:, SB * (s - 1) : SB * s]
            b1 = wp.tile([128, NHT], f32, tag="b1")
            nc.scalar.dma_start(out=b1[:], in_=b1_d[:, :])
            w2 = wp.tile([128, NHT * NCLS], bf16, tag="w2")
            nc.scalar.dma_start(out=w2[:], in_=w2_d[:, :])
            b2 = wp.tile([NCLS, 1], f32, tag="b2")
            nc.scalar.dma_start(out=b2[:], in_=b2_d[:, :])

            # ---- fold: W1_eff[q, h] = sum_m C^T[m, q] * W1[m, h] ----
            pair_idx = {pair: i for i, pair in enumerate(_PAIRS)}
            w1eff = []
            for t in range(NQT):
                ps = psF.tile([QT, HID], f32, tag="ps", name=f"foldps_{t}")
                mcs = _BLOCKS[t]
                for j, mc in enumerate(mcs):
                    rows = min(128, M26 - 128 * mc)
                    p = pair_idx[(t, mc)]
                    nc.tensor.matmul(
                        ps[:],
                        lhsT=cmb[:rows, QT * p : QT * (p + 1)],
                        rhs=w1p[:rows, HID * mc : HID * (mc + 1)],
                        start=(j == 0),
                        stop=(j == len(mcs) - 1),
                    )
                we = wp.tile([QT, HID], bf16, tag=f"we{t}", name=f"we{t}")
                if t % 2 == 0:
                    nc.vector.tensor_copy(we[:], ps[:])
                else:
                    nc.scalar.activation(we[:], ps[:], AF.Copy)
                w1eff.append(we)

            # ---- main loop over batch superblocks ----
            for s in range(NSB):
                ps1s = [
                    ps1p.tile([128, SB], f32, tag=f"ps1_{ht}", name=f"ps1_{ht}")
                    for ht in range(NHT)
                ]
                hs = []
                # ht-outer: each relu fires right after its group, so neither
                # the L2 matmuls nor the next superblock wait on a burst of
                # ACTs at the superblock boundary
                for ht in range(NHT):
                    for t in range(NQT):
                        nc.tensor.matmul(
                            ps1s[ht][:],
                            lhsT=w1eff[t][:, 128 * ht : 128 * (ht + 1)],
                            rhs=xslice(s, t),
                            start=(t == 0),
                            stop=(t == NQT - 1),
                        )
                    h = hp.tile([128, SB], bf16, tag=f"h{ht}", name=f"h{ht}")
                    nc.scalar.activation(
                        h[:],
                        ps1s[ht][:],
                        AF.Relu,
                        bias=b1[:, ht : ht + 1],
                        scale=1.0,
                    )
                    hs.append(h)
                ps2 = ps2p.tile([NCLS, SB], f32)
                for ht in range(NHT):
                    nc.tensor.matmul(
                        ps2[:],
                        lhsT=w2[:, NCLS * ht : NCLS * (ht + 1)],
                        rhs=hs[ht][:],
                        start=(ht == 0),
                        stop=(ht == NHT - 1),
                    )
                lg = lp.tile([NCLS, SB], f32, tag="lg")
                nc.vector.tensor_scalar(
                    lg[:], ps2[:], b2[:, 0:1], None, mybir.AluOpType.add
                )
                nc.sync.dma_start(out=out_d[:, SB * s : SB * (s + 1)], in_=lg[:])

    nc.compile()
    return nc


def _get_nc():
    if "nc" not in _CACHE:
        _CACHE["nc"] = _build()
    return _CACHE["nc"]


def kernel(x, conv_w, W1, b1, W2, b2):
    from concourse.bass_utils import run_bass_kernel_spmd

    nc = _get_nc()

    # C [784, 676]: y_flat = x @ C  (banded placement of conv_w values)
    C = np.zeros((Q, M26), dtype=np.float32)
    ii, jj = np.meshgrid(np.arange(OUT), np.arange(OUT), indexing="ij")
    m = (OUT * ii + jj).ravel()
    cw = np.asarray(conv_w, dtype=np.float32)
    for di in range(K):
        for dj in range(K):
            q = ((ii + di) * IMG + (jj + dj)).ravel()
            C[q, m] = cw[di, dj]
    CT = C.T  # [676, 784]
    # packed banded blocks: cmbl[p, 112*i : 112*(i+1)] = block i, row p
    cmbl = np.zeros((128, NP_ * QT), dtype=np.float32)
    for p, (t, mc) in enumerate(_PAIRS):
        rows = min(128, M26 - 128 * mc)
        cmbl[:rows, QT * p : QT * (p + 1)] = CT[
            128 * mc : 128 * mc + rows, QT * t : QT * (t + 1)
        ]
    cmbl = cmbl.astype(_BF16)

    # packed W1: w1p[p, 512*mc : 512*(mc+1)] = W1[128*mc + p, :]
    w1f = np.asarray(W1, np.float32)
    w1p = np.zeros((128, NMC * HID), dtype=np.float32)
    for mc in range(NMC):
        rows = min(128, M26 - 128 * mc)
        w1p[:rows, HID * mc : HID * (mc + 1)] = w1f[128 * mc : 128 * mc + rows, :]
    w1p = w1p.astype(_BF16)

    b1l = np.ascontiguousarray(
        np.asarray(b1, np.float32).reshape(NHT, 128).T
    )  # [128, 4]
    w2l = np.ascontiguousarray(
        np.asarray(W2, np.float32)
        .reshape(NHT, 128, NCLS)
        .transpose(1, 0, 2)
        .reshape(128, NHT * NCLS)
    ).astype(_BF16)
    b2l = np.asarray(b2, np.float32).reshape(NCLS, 1)

    xf = np.asarray(x, np.float32)
    in_maps = []
    for c in range(NCORES):
        xt = np.ascontiguousarray(xf[c * BL : (c + 1) * BL].T).astype(_BF16)
        in_maps.append(
            {
                "xt": xt,
                "cmbl": cmbl,
                "w1p": w1p,
                "b1l": b1l,
                "w2l": w2l,
                "b2l": b2l,
            }
        )

    kwargs = {}
    if TRACE:
        import profhook  # noqa: F401  (installs the NTFF hook shim)
        import tempfile

        kwargs = {"trace": True, "tmpdir": tempfile.mkdtemp(prefix="ntff_")}
    res = run_bass_kernel_spmd(nc, in_maps, core_ids=list(range(NCORES)), **kwargs)
    if TRACE:
        _CACHE["last_results"] = res

    out = np.concatenate(
        [np.ascontiguousarray(res.results[c]["out"].T) for c in range(NCORES)], axis=0
    ).astype(np.float32)
    return out


# revision 22
# speedup vs baseline: 1.0419x; 1.0103x over previous
"""Trainium2 Bass kernel for DigitConvolutionalModel (conv3x3 -> FC512 -> FC10).

Math: the 3x3 valid conv is linear, so  y_flat = x @ C  with C [784, 676]
holding conv_w values in a banded structure.  Then
    logits = relu(x @ (C @ W1) + b1) @ W2 + b2
The fold W1_eff = C @ W1 is computed on device (banded matmul over only
the nonzero blocks), then the big [2048, 784] @ [784, 512] matmul per
core, relu, and the [*, 512] @ [512, 10] head.  Data-parallel across 8
cores on the batch dim.

Host-side work is layout/dtype only: shard + transpose x, cast to bf16,
arrange C^T / W1 / b1 / W2 / b2 into SBUF-friendly layouts.
"""

import numpy as np
import ml_dtypes

B = 16384
IMG = 28
K = 3
OUT = IMG - K + 1  # 26
M26 = OUT * OUT  # 676
Q = IMG * IMG  # 784
HID = 512
NCLS = 10

NCORES = 8
BL = B // NCORES  # 2048 rows per core
QT = 112  # q-tile height (partition dim), 7 tiles
NQT = Q // QT  # 7
SB = 512  # batch superblock (matmul N)
NSB = BL // SB  # 4
NHT = HID // 128  # 4
NMC = (M26 + 127) // 128  # 6 m-chunks
NWARM = 9  # dummy matmuls to warm the PE/HAM during the DMA prologue

TRACE = False  # set by test harness to capture an NTFF profile
_CACHE = {}

_BF16 = ml_dtypes.bfloat16


def _band_blocks():
    """Static nonzero block pattern of C^T [676, 784] against (mc, qt) tiling.

    Returns per q-tile the list of m-chunk indices whose [128, QT] block of
    C^T contains structural nonzeros.
    """
    Cs = np.zeros((Q, M26), dtype=bool)
    ii, jj = np.meshgrid(np.arange(OUT), np.arange(OUT), indexing="ij")
    m = (OUT * ii + jj).ravel()
    for di in range(K):
        for dj in range(K):
            q = ((ii + di) * IMG + (jj + dj)).ravel()
            Cs[q, m] = True
    CT = Cs.T  # [676, 784]
    blocks = []
    for t in range(NQT):
        mcs = []
        for mc in range(NMC):
            rows = min(128, M26 - 128 * mc)
            if CT[128 * mc : 128 * mc + rows, QT * t : QT * (t + 1)].any():
                mcs.append(mc)
        blocks.append(mcs)
    return blocks


_BLOCKS = _band_blocks()
# flat list of (t, mc) pairs; the packed cmat input carries one [128, QT]
# block per pair, in this order
_PAIRS = [(t, mc) for t in range(NQT) for mc in _BLOCKS[t]]
NP_ = len(_PAIRS)


def _build():
    import concourse.bacc as bacc
    import concourse.mybir as mybir
    import concourse.tile as tile

    f32 = mybir.dt.float32
    bf16 = mybir.dt.bfloat16
    AF = mybir.ActivationFunctionType

    nc = bacc.Bacc("TRN2", target_bir_lowering=False, debug=False)

    # packed weights: one wide row per partition so every DMA moves
    # multi-KB contiguous chunks
    xt_d = nc.dram_tensor("xt", [Q, BL], bf16, kind="ExternalInput")
    cm_d = nc.dram_tensor("cmbl", [128, NP_ * QT], bf16, kind="ExternalInput")
    w1_d = nc.dram_tensor("w1p", [128, NMC * HID], bf16, kind="ExternalInput")
    b1_d = nc.dram_tensor("b1l", [128, NHT], f32, kind="ExternalInput")
    w2_d = nc.dram_tensor("w2l", [128, NHT * NCLS], bf16, kind="ExternalInput")
    b2_d = nc.dram_tensor("b2l", [NCLS, 1], f32, kind="ExternalInput")
    out_d = nc.dram_tensor("out", [NCLS, BL], f32, kind="ExternalOutput")

    with tile.TileContext(nc) as tc:
        with (
            tc.tile_pool(name="weights", bufs=1) as wp,
            tc.tile_pool(name="xin", bufs=1) as xp,
            tc.tile_pool(name="hid", bufs=2) as hp,
            tc.tile_pool(name="lgts", bufs=2) as lp,
            tc.tile_pool(name="psF", bufs=2, space="PSUM") as psF,
            tc.tile_pool(name="ps1", bufs=1, space="PSUM") as ps1p,
            tc.tile_pool(name="ps2", bufs=2, space="PSUM") as ps2p,
        ):
            # ---- PE warmup: dependency-light matmuls on scratch data ----
            # They issue as soon as the tiny memset lands, keeping the PE
            # busy through the weight-DMA prologue so HAM reaches K=8/8
            # before real work starts.  Results are never read.
            scratch = wp.tile([128, HID], bf16, tag="scratch")
            nc.gpsimd.memset(scratch[:], 0.0)
            warm = psF.tile([128, HID], f32, tag="ps")
            for i in range(NWARM):
                nc.tensor.matmul(
                    warm[:],
                    lhsT=scratch[:, :128],
                    rhs=scratch[:],
                    start=True,
                    stop=True,
                )

            # ---- input DMAs, spread over three descriptor paths so no
            # single queue's trigger-issue rate or ring bandwidth gates the
            # pipeline.  Each trigger occupies its queue ~650ns (HWDGE) /
            # ~1us (SWDGE), and one ring sustains only ~150-200 GB/s here.
            #   sync (HWDGE):   cmbl, x tiles for s0 and s2
            #   gpsimd (SWDGE): x tiles for s1 and s3
            #   scalar (HWDGE): w1p + small weights + logit outputs
            cmb = wp.tile([128, NP_ * QT], bf16, tag="cmb")
            nc.sync.dma_start(out=cmb[:], in_=cm_d[:, :])
            w1p = wp.tile([128, NMC * HID], bf16, tag="w1p")
            nc.scalar.dma_start(out=w1p[:], in_=w1_d[:, :])
            b1 = wp.tile([128, NHT], f32, tag="b1")
            nc.scalar.dma_start(out=b1[:], in_=b1_d[:, :])
            w2 = wp.tile([128, NHT * NCLS], bf16, tag="w2")
            nc.scalar.dma_start(out=w2[:], in_=w2_d[:, :])
            b2 = wp.tile([NCLS, 1], f32, tag="b2")
            nc.scalar.dma_start(out=b2[:], in_=b2_d[:, :])

            xtiles = {}
            for s in [0, 2]:
                for t in range(NQT):
                    xx = xp.tile([QT, SB], bf16, tag=f"x{s}_{t}", name=f"x{s}_{t}")
                    nc.sync.dma_start(
                        out=xx[:],
                        in_=xt_d[QT * t : QT * (t + 1), SB * s : SB * (s + 1)],
                    )
                    xtiles[(s, t)] = xx
            for s in [1, 3]:
                for t in range(NQT):
                    xx = xp.tile([QT, SB], bf16, tag=f"x{s}_{t}", name=f"x{s}_{t}")
                    nc.gpsimd.dma_start(
                        out=xx[:],
                        in_=xt_d[QT * t : QT * (t + 1), SB * s : SB * (s + 1)],
                    )
                    xtiles[(s, t)] = xx

            def xslice(s, t):
                return xtiles[(s, t)][:]

            # ---- fold: W1_eff[q, h] = sum_m C^T[m, q] * W1[m, h] ----
            pair_idx = {pair: i for i, pair in enumerate(_PAIRS)}
            w1eff = []
            for t in range(NQT):
                ps = psF.tile([QT, HID], f32, tag="ps", name=f"foldps_{t}")
                mcs = _BLOCKS[t]
                for j, mc in enumerate(mcs):
                    rows = min(128, M26 - 128 * mc)
                    p = pair_idx[(t, mc)]
                    nc.tensor.matmul(
                        ps[:],
                        lhsT=cmb[:rows, QT * p : QT * (p + 1)],
                        rhs=w1p[:rows, HID * mc : HID * (mc + 1)],
                        start=(j == 0),
                        stop=(j == len(mcs) - 1),
                    )
                we = wp.tile([QT, HID], bf16, tag=f"we{t}", name=f"we{t}")
                half = HID // 2
                nc.vector.tensor_copy(we[:, :half], ps[:, :half])
                nc.scalar.activation(we[:, half:], ps[:, half:], AF.Copy)
                w1eff.append(we)

            # ---- main loop over batch superblocks ----
            for s in range(NSB):
                ps1s = [
                    ps1p.tile([128, SB], f32, tag=f"ps1_{ht}", name=f"ps1_{ht}")
                    for ht in range(NHT)
                ]
                hs = []
                # ht-outer: each relu fires right after its group, so neither
                # the L2 matmuls nor the next superblock wait on a burst of
                # ACTs at the superblock boundary
                for ht in range(NHT):
                    for t in range(NQT):
                        nc.tensor.matmul(
                            ps1s[ht][:],
                            lhsT=w1eff[t][:, 128 * ht : 128 * (ht + 1)],
                            rhs=xslice(s, t),
                            start=(t == 0),
                            stop=(t == NQT - 1),
                        )
                    h = hp.tile([128, SB], bf16, tag=f"h{ht}", name=f"h{ht}")
                    nc.scalar.activation(
                        h[:],
                        ps1s[ht][:],
                        AF.Relu,
                        bias=b1[:, ht : ht + 1],
                        scale=1.0,
                    )
                    hs.append(h)
                ps2 = ps2p.tile([NCLS, SB], f32)
                for ht in range(NHT):
                    nc.tensor.matmul(
                        ps2[:],
                        lhsT=w2[:, NCLS * ht : NCLS * (ht + 1)],
                        rhs=hs[ht][:],
                        start=(ht == 0),
                        stop=(ht == NHT - 1),
                    )
                lg = lp.tile([NCLS, SB], f32, tag="lg")
                nc.vector.tensor_scalar(
                    lg[:], ps2[:], b2[:, 0:1], None, mybir.AluOpType.add
                )
                nc.sync.dma_start(out=out_d[:, SB * s : SB * (s + 1)], in_=lg[:])

    nc.compile()
    return nc


def _get_nc():
    if "nc" not in _CACHE:
        _CACHE["nc"] = _build()
    return _CACHE["nc"]


def kernel(x, conv_w, W1, b1, W2, b2):
    from concourse.bass_utils import run_bass_kernel_spmd

    nc = _get_nc()

    # C [784, 676]: y_flat = x @ C  (banded placement of conv_w values)
    C = np.zeros((Q, M26), dtype=np.float32)
    ii, jj = np.meshgrid(np.arange(OUT), np.arange(OUT), indexing="ij")
    m = (OUT * ii + jj).ravel()
    cw = np.asarray(conv_w, dtype=np.float32)
    for di in range(K):
        for dj in range(K):
            q = ((ii + di) * IMG + (jj + dj)).ravel()
            C[q, m] = cw[di, dj]
    CT = C.T  # [676, 784]
    # packed banded blocks: cmbl[p, 112*i : 112*(i+1)] = block i, row p
    cmbl = np.zeros((128, NP_ * QT), dtype=np.float32)
    for p, (t, mc) in enumerate(_PAIRS):
        rows = min(128, M26 - 128 * mc)
        cmbl[:rows, QT * p : QT * (p + 1)] = CT[
            128 * mc : 128 * mc + rows, QT * t : QT * (t + 1)
        ]
    cmbl = cmbl.astype(_BF16)

    # packed W1: w1p[p, 512*mc : 512*(mc+1)] = W1[128*mc + p, :]
    w1f = np.asarray(W1, np.float32)
    w1p = np.zeros((128, NMC * HID), dtype=np.float32)
    for mc in range(NMC):
        rows = min(128, M26 - 128 * mc)
        w1p[:rows, HID * mc : HID * (mc + 1)] = w1f[128 * mc : 128 * mc + rows, :]
    w1p = w1p.astype(_BF16)

    b1l = np.ascontiguousarray(
        np.asarray(b1, np.float32).reshape(NHT, 128).T
    )  # [128, 4]
    w2l = np.ascontiguousarray(
        np.asarray(W2, np.float32)
        .reshape(NHT, 128, NCLS)
        .transpose(1, 0, 2)
        .reshape(128, NHT * NCLS)
    ).astype(_BF16)
    b2l = np.asarray(b2, np.float32).reshape(NCLS, 1)

    xf = np.asarray(x, np.float32)
    in_maps = []
    for c in range(NCORES):
        xt = np.ascontiguousarray(xf[c * BL : (c + 1) * BL].T).astype(_BF16)
        in_maps.append(
            {
                "xt": xt,
                "cmbl": cmbl,
                "w1p": w1p,
                "b1l": b1l,
                "w2l": w2l,
                "b2l": b2l,
            }
        )

    kwargs = {}
    if TRACE:
        import profhook  # noqa: F401  (installs the NTFF hook shim)
        import tempfile

        kwargs = {"trace": True, "tmpdir": tempfile.mkdtemp(prefix="ntff_")}
    res = run_bass_kernel_spmd(nc, in_maps, core_ids=list(range(NCORES)), **kwargs)
    if TRACE:
        _CACHE["last_results"] = res

    out = np.concatenate(
        [np.ascontiguousarray(res.results[c]["out"].T) for c in range(NCORES)], axis=0
    ).astype(np.float32)
    return out
